# revision 1
# baseline (speedup 1.0000x reference)
"""GeomGCN (2-layer relational GCN) distributed Bass kernel for 8 TRN2 NeuronCores.

Strategy (source-sharded, graph-parallel):
  - Nodes are split into 8 contiguous slices of NLOC; core k owns slice k.
  - Math: with dinv = deg^-1/2 (deg = in-degree by `row` over all edges),
      h1 = sum_r A_r (dinv*x) @ W1_r  scaled by dinv on the dest side, + b1
    Each core computes y1 = (dinv*x_k) @ W1_r for its node slice (dense
    matmuls), stores a per-relation message table in DRAM, then gathers
    per-edge rows with dma_gather (edges assigned to the core that owns the
    edge's source node `col`).  Edges are host-sorted by destination and
    padded into 128-edge chunks per 128-dest tile; a one-hot selection
    matrix (built on-device via iota/is_equal) turns the per-tile
    segment-sum into TensorE matmuls accumulated in PSUM.  Per-node partial
    sums [N_PAD, H] are combined with a ReduceScatter so each core ends up
    with its own node slice of h1; layer 2 repeats the same pattern with
    16-wide messages, then log_softmax.
  Host work is index-only: slicing, sorting, degree counts, chunk/slot
  assignment and int16 gather-index tables.
"""
import math
import os
import numpy as np

import concourse.bass as bass
import concourse.tile as tile
from concourse import bacc, mybir
from concourse.bass_utils import run_bass_kernel_spmd

F32 = mybir.dt.float32
I16 = mybir.dt.int16
AF = mybir.ActivationFunctionType
ALU = mybir.AluOpType


class Cfg:
    def __init__(self, N, E, F, H, C, R, ncores=8, B=64, J=8):
        self.N, self.E, self.F, self.H, self.C, self.R = N, E, F, H, C, R
        self.ncores = ncores
        self.P = 128
        # node slice per core, padded so N_PAD = ncores * NLOC and NLOC
        # covers the largest slice
        self.NLOC = math.ceil(math.ceil(N / ncores) / 16) * 16
        while (self.NLOC * ncores) % 128 != 0:
            self.NLOC += 16
        self.N_PAD = self.NLOC * ncores
        self.NT = self.N_PAD // 128                  # dest tiles
        self.MC = math.ceil(self.NLOC / 128)         # m-chunks per slice
        self.MPAD = self.MC * 128
        self.YSTRIDE = self.MPAD + 128               # rows per relation in table
        self.DUMMY = self.MPAD                       # a guaranteed-zero table row
        self.E1 = H                                  # layer-1 message width
        self.E2 = max(64, C)                         # layer-2 row (256B min)
        self.B = B                                   # gather chunks per batch
        self.J = J                                   # S^T chunks per build
        self.KC = F // 128                           # k-chunks layer-1 dense
        assert F % 128 == 0 and H == 128 and self.YSTRIDE % 16 == 0
        assert self.R * self.YSTRIDE < 32768, "int16 gather index overflow"


CFG = Cfg(N=50000, E=800000, F=256, H=128, C=16, R=4, B=8)


# ----------------------------------------------------------------- host side
def preprocess(cfg, x, edge_index, edge_relation, W1, b1, W2, b2):
    N, ncores, NLOC, NT = cfg.N, cfg.ncores, cfg.NLOC, cfg.NT
    row = np.asarray(edge_index[0], dtype=np.int64)
    col = np.asarray(edge_index[1], dtype=np.int64)
    rel = np.asarray(edge_relation, dtype=np.int64)
    x = np.asarray(x, dtype=np.float32)

    deg = np.bincount(row, minlength=N).astype(np.float32)

    # per-core edge sets (by source/col ownership), sorted by dest row
    per_core = []
    counts = np.zeros((ncores, NT), dtype=np.int64)
    for k in range(ncores):
        m = (col // NLOC) == k
        er, ec, eg = row[m], col[m] - k * NLOC, rel[m]
        o = np.argsort(er, kind="stable")
        er, ec, eg = er[o], ec[o], eg[o]
        t = er // 128
        counts[k] = np.bincount(t, minlength=NT)
        per_core.append((er, ec, eg, t))

    # chunks per dest tile = max over cores (>=1), shared static schedule
    chunks_t = np.maximum(1, np.ceil(counts.max(axis=0) / 128).astype(np.int64))
    CH = int(chunks_t.sum())
    CHpad = math.ceil(CH / cfg.B) * cfg.B
    NB = CHpad // cfg.B
    slot_base = np.concatenate([[0], np.cumsum(chunks_t * 128)])[:-1]

    in_maps = []
    iota = np.broadcast_to(np.arange(128, dtype=np.float32), (128, 128)).copy()
    ident = np.eye(128, dtype=np.float32)
    for k in range(ncores):
        er, ec, eg, t = per_core[k]
        first = np.searchsorted(t, np.arange(NT), side="left")
        rank = np.arange(len(t)) - first[t]
        slots = slot_base[t] + rank
        gidx = np.full(CHpad * 128, cfg.DUMMY, dtype=np.int16)
        gidx[slots] = (eg * cfg.YSTRIDE + ec).astype(np.int16)
        dloc = np.zeros(CHpad * 128, dtype=np.float32)
        dloc[slots] = (er % 128).astype(np.float32)

        # wrapped-16 int16 index layout per batch, replicated to 8 groups
        g = gidx.reshape(NB, cfg.B * 8, 16)              # [b, s, r]
        w = np.transpose(g, (0, 2, 1))                   # [b, r, s]
        gidx_w = np.broadcast_to(
            w[:, None, :, :], (NB, 8, 16, cfg.B * 8)
        ).reshape(NB * 128, cfg.B * 8).astype(np.int16)
        dloc_w = np.ascontiguousarray(
            dloc.reshape(CHpad, 128).T
        ).reshape(128, CHpad, 1)

        lo = k * NLOC
        hi = min(N, lo + NLOC)
        xk = np.zeros((cfg.MPAD, cfg.F), dtype=np.float32)
        xk[: hi - lo] = x[lo:hi]
        dk = np.zeros(cfg.MPAD, dtype=np.float32)
        dk[: hi - lo] = deg[lo:hi]

        in_maps.append({
            "xT": np.ascontiguousarray(xk.T),
            "degc": np.ascontiguousarray(dk.reshape(cfg.MC, 128).T),
            "W1": np.asarray(W1, dtype=np.float32),
            "W2": np.asarray(W2, dtype=np.float32),
            "b1c": np.asarray(b1, dtype=np.float32).reshape(cfg.H, 1),
            "b2r": np.broadcast_to(np.asarray(b2, dtype=np.float32),
                                   (128, cfg.C)).copy(),
            "iota": iota.reshape(128, 1, 128),
            "ident": ident,
            "gidx": gidx_w,
            "dloc": dloc_w,
        })
    return in_maps, tuple(int(v) for v in chunks_t), CHpad


# --------------------------------------------------------------- device side
def build_program(cfg, chunks_t, CHpad):
    P, R, H, C = cfg.P, cfg.R, cfg.H, cfg.C
    NB = CHpad // cfg.B
    nc = bacc.Bacc("TRN2", target_bir_lowering=False, debug=False,
                   num_devices=cfg.ncores)

    xT = nc.dram_tensor("xT", [cfg.F, cfg.MPAD], F32, kind="ExternalInput").ap()
    degc = nc.dram_tensor("degc", [128, cfg.MC], F32, kind="ExternalInput").ap()
    W1 = nc.dram_tensor("W1", [R * cfg.F, H], F32, kind="ExternalInput").ap()
    W2 = nc.dram_tensor("W2", [R * H, C], F32, kind="ExternalInput").ap()
    b1c = nc.dram_tensor("b1c", [H, 1], F32, kind="ExternalInput").ap()
    b2r = nc.dram_tensor("b2r", [128, C], F32, kind="ExternalInput").ap()
    iota = nc.dram_tensor("iota", [128, 1, 128], F32, kind="ExternalInput").ap()
    ident = nc.dram_tensor("ident", [128, 128], F32, kind="ExternalInput").ap()
    gidx = nc.dram_tensor("gidx", [NB * 128, cfg.B * 8], I16,
                          kind="ExternalInput").ap()
    dloc = nc.dram_tensor("dloc", [128, CHpad, 1], F32,
                          kind="ExternalInput").ap()
    out = nc.dram_tensor("out", [cfg.NLOC, C], F32, kind="ExternalOutput").ap()

    with tile.TileContext(nc) as tc:
        _build(tc, cfg, chunks_t, CHpad, xT, degc, W1, W2, b1c, b2r,
               iota, ident, gidx, dloc, out)
    nc.compile()
    return nc


def _build(tc, cfg, chunks_t, CHpad, xT, degc, W1, W2, b1c, b2r,
           iota, ident, gidx, dloc, out):
    nc = tc.nc
    P, R, H, C = cfg.P, cfg.R, cfg.H, cfg.C
    B, J, MC, NT = cfg.B, cfg.J, cfg.MC, cfg.NT
    NB = CHpad // B
    with tc.tile_pool(name="const", bufs=1) as cpool, \
         tc.tile_pool(name="big", bufs=1) as bigp, \
         tc.tile_pool(name="gY", bufs=3) as gpool, \
         tc.tile_pool(name="idx", bufs=3) as ipool, \
         tc.tile_pool(name="s3", bufs=3) as spool, \
         tc.tile_pool(name="stage", bufs=6) as stpool, \
         tc.tile_pool(name="psum", bufs=6, space="PSUM") as pp, \
         tc.tile_pool(name="dram", bufs=1, space="DRAM") as dram:

        # ---------- constants / degree scaling
        iota_t = cpool.tile([128, 1, 128], F32)
        nc.sync.dma_start(out=iota_t[:], in_=iota[:, :, :])
        ident_t = cpool.tile([128, 128], F32)
        nc.sync.dma_start(out=ident_t[:], in_=ident[:, :])
        b2_t = cpool.tile([128, C], F32)
        nc.sync.dma_start(out=b2_t[:], in_=b2r[:, :])
        b1_t = cpool.tile([H, 1], F32)
        nc.sync.dma_start(out=b1_t[:], in_=b1c[:, :])
        w1b = {}
        for r in range(R):
            for kc in range(cfg.KC):
                t = cpool.tile([128, H], F32, tag=f"w1_{r}_{kc}")
                nc.sync.dma_start(
                    out=t[:], in_=W1[r * cfg.F + kc * 128:
                                     r * cfg.F + (kc + 1) * 128, :])
                w1b[(r, kc)] = t
        w2b = {}
        for r in range(R):
            t = cpool.tile([H, C], F32, tag=f"w2_{r}")
            nc.sync.dma_start(out=t[:], in_=W2[r * H:(r + 1) * H, :])
            w2b[r] = t

        def make_dinv(src_ap, shape, tag):
            d = cpool.tile(shape, F32, tag=f"deg_{tag}")
            nc.sync.dma_start(out=d[:], in_=src_ap)
            mask = cpool.tile(shape, F32, tag=f"m_{tag}")
            nc.vector.tensor_scalar(out=mask[:], in0=d[:], scalar1=0.0,
                                    scalar2=None, op0=ALU.is_gt)
            s = cpool.tile(shape, F32, tag=f"s_{tag}")
            nc.scalar.sqrt(out=s[:], in_=d[:])
            rcp = cpool.tile(shape, F32, tag=f"r_{tag}")
            nc.vector.reciprocal(out=rcp[:], in_=s[:])
            dv = cpool.tile(shape, F32, tag=f"dv_{tag}")
            nc.vector.tensor_mul(out=dv[:], in0=rcp[:], in1=mask[:])
            return dv

        dinv_c = make_dinv(degc[:, :], [128, MC], "c")         # col layout
        dinv2_c = cpool.tile([128, MC], F32)
        nc.vector.tensor_mul(out=dinv2_c[:], in0=dinv_c[:], in1=dinv_c[:])

        # ---------- layer-1 dense: y1[r*YS + m, :] = (dinv*x)[m] @ W1_r
        uT = {}
        for kc in range(cfg.KC):
            t = bigp.tile([128, cfg.MPAD], F32, tag=f"uT{kc}")
            nc.sync.dma_start(out=t[:], in_=xT[kc * 128:(kc + 1) * 128, :])
            uT[kc] = t

        y1_dram = dram.tile([R * cfg.YSTRIDE, H], F32)
        for r in range(R):
            for mc in range(MC):
                ps = pp.tile([128, H], F32, tag="ps")
                for kc in range(cfg.KC):
                    nc.tensor.matmul(
                        out=ps[:],
                        lhsT=uT[kc][:, mc * 128:(mc + 1) * 128],
                        rhs=w1b[(r, kc)][:],
                        start=(kc == 0), stop=(kc == cfg.KC - 1))
                st = stpool.tile([128, H], F32, tag="ev1")
                nc.scalar.mul(out=st[:], in_=ps[:], mul=dinv_c[:, mc:mc + 1])
                nc.sync.dma_start(
                    out=y1_dram[r * cfg.YSTRIDE + mc * 128:
                                r * cfg.YSTRIDE + (mc + 1) * 128, :],
                    in_=st[:])
        # zero the dummy rows (row DUMMY..DUMMY+127 per relation)
        zt = cpool.tile([128, H], F32)
        nc.vector.memset(zt[:], 0.0)
        for r in range(R):
            nc.sync.dma_start(
                out=y1_dram[r * cfg.YSTRIDE + cfg.DUMMY:
                            r * cfg.YSTRIDE + cfg.DUMMY + 128, :],
                in_=zt[:])

        LIMIT = int(os.environ.get("KLIMIT", "6"))
        if LIMIT < 2:
            return
        # ---------- shared: dest-local ids for one-hot building
        dloc_t = bigp.tile([128, CHpad, 1], F32)
        nc.sync.dma_start(out=dloc_t[:], in_=dloc[:, :, :])

        def agg_pass(table_ap, elem, width, part_dram, tagsfx):
            """Gather + one-hot matmul segment sum; stream per-tile results."""
            batches = {}

            def batch(b):
                if b not in batches:
                    it = ipool.tile([128, B * 8], I16, tag="idx")
                    nc.sync.dma_start(
                        out=it[:], in_=gidx[b * 128:(b + 1) * 128, :])
                    g = gpool.tile([128, B, elem], F32, tag="g")
                    nc.gpsimd.dma_gather(
                        out_ap=g[:], in_ap=table_ap, idxs_ap=it[:],
                        num_idxs=B * 128, num_idxs_reg=B * 128,
                        elem_size=elem)
                    batches[b] = g
                return batches[b]

            s3 = None
            c = 0
            for t in range(NT):
                ps = pp.tile([128, width], F32, tag="ps")
                for j in range(chunks_t[t]):
                    g = batch(c // B)
                    if c % J == 0:
                        s3 = spool.tile([128, J, 128], F32, tag="s3")
                        nj = min(J, CHpad - c)
                        nc.vector.tensor_tensor(
                            out=s3[:, :nj, :],
                            in0=dloc_t[:, c:c + nj, :].to_broadcast(
                                [128, nj, 128]),
                            in1=iota_t[:].to_broadcast([128, nj, 128]),
                            op=ALU.is_equal)
                    nc.tensor.matmul(
                        out=ps[:], lhsT=s3[:, c % J, :],
                        rhs=g[:, c % B, :width],
                        start=(j == 0), stop=(j == chunks_t[t] - 1))
                    c += 1
                st = stpool.tile([128, width], F32, tag=f"ev{tagsfx}")
                nc.scalar.copy(out=st[:], in_=ps[:])
                nc.sync.dma_start(
                    out=part_dram[t * 128:(t + 1) * 128, :], in_=st[:])

        # ---------- layer-1 aggregation + reduce-scatter
        t1_part = dram.tile([cfg.N_PAD, H], F32)
        t1_red = dram.tile([cfg.NLOC, H], F32)
        agg_pass(y1_dram[:, :], H, H, t1_part, "1")
        if LIMIT < 3:
            return
        nc.gpsimd.collective_compute(
            "ReduceScatter", ALU.add,
            replica_groups=[list(range(cfg.ncores))],
            ins=[t1_part.opt()], outs=[t1_red.opt()])

        if LIMIT < 4:
            return
        # ---------- layer-2 dense: y2[r*YS+m, :C] = u2[m] @ W2_r + dinv[m]*c_r
        # u2 = dinv^2 * t1 + dinv * b1  (h1 = dinv*t1 + b1 folded in)
        ones_t = cpool.tile([1, 128], F32)
        nc.vector.memset(ones_t[:], 1.0)
        crow = {}
        for r in range(R):
            ps = pp.tile([1, C], F32, tag="ps")
            nc.tensor.matmul(out=ps[:], lhsT=b1_t[:], rhs=w2b[r][:],
                             start=True, stop=True)
            ct = cpool.tile([1, C], F32, tag=f"c_{r}")
            nc.scalar.copy(out=ct[:], in_=ps[:])
            psb = pp.tile([128, C], F32, tag="ps")
            nc.tensor.matmul(out=psb[:], lhsT=ones_t[:], rhs=ct[:],
                             start=True, stop=True)
            cb = cpool.tile([128, C], F32, tag=f"cb_{r}")
            nc.scalar.copy(out=cb[:], in_=psb[:])
            crow[r] = cb

        u2T = bigp.tile([128, cfg.MPAD], F32)
        nrows_last = cfg.NLOC - (MC - 1) * 128
        for mc in range(MC):
            tt = stpool.tile([128, H], F32, tag="tt")
            rows = 128 if mc < MC - 1 else nrows_last
            if rows < 128:
                nc.vector.memset(tt[:], 0.0)
            nc.sync.dma_start(out=tt[:rows, :],
                              in_=t1_red[mc * 128:mc * 128 + rows, :])
            nc.vector.tensor_scalar(out=tt[:], in0=tt[:],
                                    scalar1=dinv2_c[:, mc:mc + 1],
                                    scalar2=None, op0=ALU.mult)
            ps = pp.tile([128, 128], F32, tag="ps")
            nc.tensor.transpose(out=ps[:], in_=tt[:], identity=ident_t[:])
            nc.scalar.copy(out=u2T[:, mc * 128:(mc + 1) * 128], in_=ps[:])

        y2_dram = dram.tile([R * cfg.YSTRIDE, cfg.E2], F32)
        for r in range(R):
            for mc in range(MC):
                ps = pp.tile([128, C], F32, tag="ps")
                nc.tensor.matmul(out=ps[:],
                                 lhsT=u2T[:, mc * 128:(mc + 1) * 128],
                                 rhs=w2b[r][:], start=True, stop=True)
                st = stpool.tile([128, cfg.E2], F32, tag="ev2w")
                nc.vector.memset(st[:], 0.0)
                nc.vector.tensor_scalar(
                    out=st[:, :C], in0=crow[r][:],
                    scalar1=dinv_c[:, mc:mc + 1], scalar2=None, op0=ALU.mult)
                nc.vector.tensor_tensor(out=st[:, :C], in0=st[:, :C],
                                        in1=ps[:], op=ALU.add)
                nc.sync.dma_start(
                    out=y2_dram[r * cfg.YSTRIDE + mc * 128:
                                r * cfg.YSTRIDE + (mc + 1) * 128, :],
                    in_=st[:])
        zt2 = cpool.tile([128, cfg.E2], F32)
        nc.vector.memset(zt2[:], 0.0)
        for r in range(R):
            nc.sync.dma_start(
                out=y2_dram[r * cfg.YSTRIDE + cfg.DUMMY:
                            r * cfg.YSTRIDE + cfg.DUMMY + 128, :],
                in_=zt2[:])

        if LIMIT < 5:
            return
        # ---------- layer-2 aggregation + reduce-scatter
        t2_part = dram.tile([cfg.N_PAD, C], F32)
        t2_red = dram.tile([cfg.NLOC, C], F32)
        agg_pass(y2_dram[:, :], cfg.E2, C, t2_part, "2")
        nc.gpsimd.collective_compute(
            "ReduceScatter", ALU.add,
            replica_groups=[list(range(cfg.ncores))],
            ins=[t2_part.opt()], outs=[t2_red.opt()])

        if LIMIT < 6:
            return
        # ---------- final: h2 = dinv*t2 + b2 ; log_softmax rows
        for mc in range(MC):
            rows = 128 if mc < MC - 1 else nrows_last
            ft = stpool.tile([128, C], F32, tag="fin")
            nc.sync.dma_start(out=ft[:rows, :],
                              in_=t2_red[mc * 128:mc * 128 + rows, :])
            nc.vector.tensor_scalar(out=ft[:], in0=ft[:],
                                    scalar1=dinv_c[:, mc:mc + 1],
                                    scalar2=None, op0=ALU.mult)
            nc.vector.tensor_tensor(out=ft[:], in0=ft[:],
                                    in1=b2_t[:], op=ALU.add)
            negmx = stpool.tile([128, 1], F32, tag="mx")
            nc.vector.tensor_reduce(out=negmx[:], in_=ft[:],
                                    axis=mybir.AxisListType.X,
                                    op=ALU.max, negate=True)
            ex = stpool.tile([128, C], F32, tag="ex")
            ssum = stpool.tile([128, 1], F32, tag="sm")
            nc.scalar.activation(out=ex[:], in_=ft[:], func=AF.Exp,
                                 bias=negmx[:, 0:1], scale=1.0,
                                 accum_out=ssum[:, 0:1])
            lg = stpool.tile([128, 1], F32, tag="lg")
            nc.scalar.activation(out=lg[:], in_=ssum[:], func=AF.Ln)
            nc.vector.tensor_scalar(out=ft[:], in0=ft[:],
                                    scalar1=negmx[:, 0:1],
                                    scalar2=lg[:, 0:1],
                                    op0=ALU.add, op1=ALU.subtract)
            nc.sync.dma_start(out=out[mc * 128:mc * 128 + rows, :],
                              in_=ft[:rows, :])


# ------------------------------------------------------------------ runtime
_PROGRAM_CACHE = {}


def run(cfg, inputs):
    in_maps, chunks_t, CHpad = preprocess(cfg, **inputs)
    key = (cfg.N, cfg.E, chunks_t, CHpad)
    if key not in _PROGRAM_CACHE:
        _PROGRAM_CACHE[key] = build_program(cfg, chunks_t, CHpad)
    nc = _PROGRAM_CACHE[key]
    res = None
    for attempt in range(3):
        try:
            res = run_bass_kernel_spmd(nc, in_maps,
                                       core_ids=list(range(cfg.ncores)))
            break
        except Exception:
            if attempt == 2:
                raise
    outs = [res.results[k]["out"][:cfg.NLOC] for k in range(cfg.ncores)]
    full = np.concatenate(outs, axis=0)[:cfg.N]
    return np.ascontiguousarray(full.astype(np.float32))


def kernel(x, edge_index, edge_relation, W1, b1, W2, b2):
    return run(CFG, dict(x=x, edge_index=edge_index,
                         edge_relation=edge_relation,
                         W1=W1, b1=b1, W2=W2, b2=b2))



# revision 6
# speedup vs baseline: 10.3849x; 10.3849x over previous
"""GeomGCN (2-layer relational GCN) distributed Bass kernel for 8 TRN2 NeuronCores.

v2 strategy (node-sharded, graph-parallel, bf16 datapath):
  - Nodes split into 8 slices; core k owns slice k (both as source and dest).
  - Within each slice, dest nodes are permuted host-side (degree-sorted snake
    over TPS tiles of 128) to flatten per-(core, dest-tile) edge counts, which
    minimizes the number of 128-edge chunks (and hence gather descriptors,
    one-hot builds and segment matmuls).
  - Layer math: y1[r,src] = dinv[src]*(x[src] @ W1_r)  (dense bf16 matmuls),
    stored as a DRAM table with interleaved rows (src*R + r), 256B each.
    Edges (sorted by dest, chunked 128/dest-tile) gather their rows with
    dma_gather; a one-hot matrix (built on DVE via broadcast is_equal against
    an iota) turns the per-tile segment-sum into TensorE matmuls in PSUM.
    Pad slots carry a sentinel dest (200) so their one-hot column is all-zero:
    no dummy table rows needed.  Per-node partials are ReduceScattered (bf16);
    layer 2 repeats with 16-wide messages stored in the low 16 columns of
    another interleaved 256B-row table (same gather indices), then a fused
    full-width log_softmax.
  - All DMAs are batched (staged writes of 8-16 tiles, single table loads,
    resident index tables) to amortize the ~650ns fixed per-DMA dispatch cost.
"""
import math
import os
import numpy as np

import concourse.bass as bass
import concourse.tile as tile
from concourse import bacc, mybir
from concourse.bass_utils import run_bass_kernel_spmd
from concourse.masks import make_identity

F32 = mybir.dt.float32
BF16 = mybir.dt.bfloat16
I16 = mybir.dt.int16
BF_NP = mybir.dt.np(mybir.dt.bfloat16)
AF = mybir.ActivationFunctionType
ALU = mybir.AluOpType


class Cfg:
    def __init__(self, N, E, F, H, C, R, ncores=8, B=32, J=8):
        self.N, self.E, self.F, self.H, self.C, self.R = N, E, F, H, C, R
        self.ncores = ncores
        self.NSL = math.ceil(N / ncores)             # real nodes per slice
        tps_nodes = math.ceil(self.NSL / 128)
        tps_edges = math.ceil(E / (ncores * ncores) / 224)
        self.TPS = max(tps_nodes, tps_edges)         # dest tiles per slice
        self.NLOC = self.TPS * 128                   # padded nodes per slice
        self.MC = self.TPS                           # m-chunks per slice
        self.N_PAD = ncores * self.NLOC
        self.NT = ncores * self.TPS                  # global dest tiles
        self.KC = F // 128
        self.B = B                                   # gather chunks per batch
        self.J = J                                   # chunks per one-hot build
        self.GT1 = 8                                 # agg1 tiles per staged DMA
        self.GT2 = 16                                # agg2 tiles per staged DMA
        assert F % 128 == 0 and H == 128
        assert R * self.NLOC < 32768, "int16 gather index overflow"
        assert self.NT % self.GT1 == 0 and self.NT % self.GT2 == 0
        assert math.ceil(self.NSL / self.TPS) <= 128


CFG = Cfg(N=50000, E=800000, F=256, H=128, C=16, R=4)


# ----------------------------------------------------------------- host side
def preprocess(cfg, x, edge_index, edge_relation, W1, b1, W2, b2):
    N, nc8 = cfg.N, cfg.ncores
    NSL, NLOC, TPS, NT, MC, R, B = (cfg.NSL, cfg.NLOC, cfg.TPS, cfg.NT,
                                    cfg.MC, cfg.R, cfg.B)
    row = np.asarray(edge_index[0], dtype=np.int64)
    col = np.asarray(edge_index[1], dtype=np.int64)
    rel = np.asarray(edge_relation, dtype=np.int64)
    x = np.asarray(x, dtype=np.float32)

    deg = np.bincount(row, minlength=N).astype(np.float32)

    # per-slice balancing permutation: degree-sorted snake over TPS tiles
    newloc = np.empty(N, dtype=np.int64)
    for j in range(nc8):
        lo = j * NSL
        hi = min(N, lo + NSL)
        n = hi - lo
        order = np.argsort(-deg[lo:hi], kind="stable")
        rr = np.arange(n)
        rnd, idx = rr // TPS, rr % TPS
        tile_i = np.where(rnd % 2 == 0, idx, TPS - 1 - idx)
        pos = tile_i * 128 + rnd
        nl = np.empty(n, dtype=np.int64)
        nl[order] = pos
        newloc[lo:hi] = nl
    cfg.newloc = newloc

    er = np.minimum(row // NSL, nc8 - 1) * NLOC + newloc[row]  # new dest id
    ksrc = np.minimum(col // NSL, nc8 - 1)
    ecl = newloc[col]                                          # new src local

    counts = np.zeros((nc8, NT), dtype=np.int64)
    percore = []
    for k in range(nc8):
        m = ksrc == k
        erk, eck, egk = er[m], ecl[m], rel[m]
        o = np.argsort(erk, kind="stable")
        erk, eck, egk = erk[o], eck[o], egk[o]
        t = erk >> 7
        counts[k] = np.bincount(t, minlength=NT)
        percore.append((erk, eck, egk, t))

    chunks_t = np.maximum(1, np.ceil(counts.max(axis=0) / 128).astype(np.int64))
    CH = int(chunks_t.sum())
    CHpad = math.ceil(CH / B) * B
    NB = CHpad // B
    slot_base = np.concatenate([[0], np.cumsum(chunks_t * 128)])[:-1]

    W1b = np.asarray(W1, dtype=np.float32).astype(BF_NP)
    W2cat = (np.asarray(W2, dtype=np.float32)
             .reshape(R, cfg.H, cfg.C).transpose(1, 0, 2)
             .reshape(cfg.H, R * cfg.C).astype(BF_NP))
    b1c = np.asarray(b1, dtype=np.float32).reshape(cfg.H, 1)
    b2r = np.broadcast_to(np.asarray(b2, dtype=np.float32),
                          (128, cfg.C)).copy()

    in_maps = []
    for k in range(nc8):
        erk, eck, egk, t = percore[k]
        first = np.searchsorted(t, np.arange(NT), side="left")
        rank = np.arange(len(t)) - first[t]
        slots = slot_base[t] + rank
        gidx = np.zeros(CHpad * 128, dtype=np.int16)
        gidx[slots] = (eck * R + egk).astype(np.int16)
        dloc = np.full(CHpad * 128, 200.0, dtype=np.float32)
        dloc[slots] = (erk % 128).astype(np.float32)

        # wrapped-16 index layout, compact (replicated to 128 on device):
        # slot i of batch b -> partition i%16, free column i//16
        gw = np.ascontiguousarray(
            gidx.reshape(NB, B * 8, 16).transpose(2, 0, 1)  # [16, NB, B*8]
        ).reshape(16, NB * B * 8)
        dloc_w = np.ascontiguousarray(
            dloc.reshape(CHpad, 128).T).astype(BF_NP)        # [128, CHpad]

        lo = k * NSL
        hi = min(N, lo + NSL)
        xk = np.zeros((NLOC, cfg.F), dtype=np.float32)
        xk[newloc[lo:hi]] = x[lo:hi]
        xTb = np.ascontiguousarray(xk.T).astype(BF_NP)       # [F, NLOC]
        dk = np.zeros(NLOC, dtype=np.float32)
        dk[newloc[lo:hi]] = deg[lo:hi]
        degc = np.ascontiguousarray(dk.reshape(MC, 128).T)   # [128, MC]

        in_maps.append({
            "xT": xTb,
            "degc": degc,
            "W1": W1b,
            "W2c": W2cat,
            "b1c": b1c,
            "b2r": b2r,
            "gidx": gw,
            "dloc": dloc_w,
        })
    return in_maps, tuple(int(v) for v in chunks_t), CHpad


def assemble(cfg, outs):
    """Un-permute per-core outputs into the full [N, C] array."""
    full = np.empty((cfg.N, cfg.C), dtype=np.float32)
    for j in range(cfg.ncores):
        lo = j * cfg.NSL
        hi = min(cfg.N, lo + cfg.NSL)
        full[lo:hi] = outs[j][cfg.newloc[lo:hi]]
    return full


# --------------------------------------------------------------- device side
def build_program(cfg, chunks_t, CHpad):
    R, H, C, F = cfg.R, cfg.H, cfg.C, cfg.F
    NB = CHpad // cfg.B
    nc = bacc.Bacc("TRN2", target_bir_lowering=False, debug=False,
                   num_devices=cfg.ncores)

    xT = nc.dram_tensor("xT", [F, cfg.NLOC], BF16, kind="ExternalInput").ap()
    degc = nc.dram_tensor("degc", [128, cfg.MC], F32, kind="ExternalInput").ap()
    W1 = nc.dram_tensor("W1", [R * F, H], BF16, kind="ExternalInput").ap()
    W2c = nc.dram_tensor("W2c", [H, R * C], BF16, kind="ExternalInput").ap()
    b1c = nc.dram_tensor("b1c", [H, 1], F32, kind="ExternalInput").ap()
    b2r = nc.dram_tensor("b2r", [128, C], F32, kind="ExternalInput").ap()
    gidx = nc.dram_tensor("gidx", [16, NB * cfg.B * 8], I16,
                          kind="ExternalInput").ap()
    dloc = nc.dram_tensor("dloc", [128, CHpad], BF16, kind="ExternalInput").ap()
    out = nc.dram_tensor("out", [cfg.NLOC, C], F32, kind="ExternalOutput").ap()

    with tile.TileContext(nc) as tc:
        _build(tc, cfg, chunks_t, CHpad, xT, degc, W1, W2c, b1c, b2r,
               gidx, dloc, out)
    nc.compile()
    return nc


def _build(tc, cfg, chunks_t, CHpad, xT, degc, W1, W2c, b1c, b2r,
           gidx, dloc, out):
    nc = tc.nc
    R, H, C = cfg.R, cfg.H, cfg.C
    B, J, MC, NT, KC = cfg.B, cfg.J, cfg.MC, cfg.NT, cfg.KC
    NB = CHpad // B
    RC = R * C
    B8 = B * 8
    with tc.tile_pool(name="const", bufs=1) as cpool, \
         tc.tile_pool(name="big", bufs=1) as bigp, \
         tc.tile_pool(name="gY", bufs=3) as gpool, \
         tc.tile_pool(name="s3", bufs=3) as spool, \
         tc.tile_pool(name="stage", bufs=4) as stpool, \
         tc.tile_pool(name="psum", bufs=6, space="PSUM") as pp, \
         tc.tile_pool(name="dram", bufs=1, space="DRAM") as dram:

        # ---------- constants
        iota16 = cpool.tile([128, 128], I16)
        nc.gpsimd.iota(iota16[:], pattern=[[1, 128]], base=0,
                       channel_multiplier=0)
        iotab = cpool.tile([128, 1, 128], BF16)
        nc.vector.tensor_copy(out=iotab[:, 0, :], in_=iota16[:])
        identf = cpool.tile([128, 128], F32)
        make_identity(nc, identf[:])
        identb = cpool.tile([128, 128], BF16)
        nc.vector.tensor_copy(out=identb[:], in_=identf[:])
        b2t = cpool.tile([128, C], F32)
        nc.sync.dma_start(out=b2t[:], in_=b2r[:, :])
        b1t = cpool.tile([H, 1], F32)
        nc.sync.dma_start(out=b1t[:], in_=b1c[:, :])
        b1tb = cpool.tile([H, 1], BF16)
        nc.vector.tensor_copy(out=b1tb[:], in_=b1t[:])
        w2t = cpool.tile([H, RC], BF16)
        nc.sync.dma_start(out=w2t[:], in_=W2c[:, :])
        w1t = cpool.tile([128, R * KC, H], BF16)
        nc.sync.dma_start(out=w1t[:],
                          in_=W1.rearrange("(q p) h -> p q h", p=128))

        degt = cpool.tile([128, MC], F32)
        nc.sync.dma_start(out=degt[:], in_=degc[:, :])
        dmask = cpool.tile([128, MC], F32)
        nc.vector.tensor_scalar(out=dmask[:], in0=degt[:], scalar1=0.0,
                                scalar2=None, op0=ALU.is_gt)
        dsq = cpool.tile([128, MC], F32)
        nc.scalar.sqrt(out=dsq[:], in_=degt[:])
        drcp = cpool.tile([128, MC], F32)
        nc.vector.reciprocal(out=drcp[:], in_=dsq[:])
        dinv = cpool.tile([128, MC], F32)
        nc.vector.tensor_mul(out=dinv[:], in0=drcp[:], in1=dmask[:])
        dinv2 = cpool.tile([128, MC], F32)
        nc.vector.tensor_mul(out=dinv2[:], in0=dinv[:], in1=dinv[:])

        # resident gather-index table, replicated 16 -> 128 partitions
        idxt = bigp.tile([128, NB * B8], I16)
        nc.sync.dma_start(out=idxt[0:16, :], in_=gidx[:, :])
        nc.sync.dma_start(out=idxt[16:32, :], in_=idxt[0:16, :])
        nc.sync.dma_start(out=idxt[32:64, :], in_=idxt[0:32, :])
        nc.sync.dma_start(out=idxt[64:128, :], in_=idxt[0:64, :])
        dloct = bigp.tile([128, CHpad, 1], BF16)
        nc.sync.dma_start(out=dloct[:, :, 0], in_=dloc[:, :])

        # crow[r*C+c] = b1 @ W2_r, replicated to 128 partitions
        psc = pp.tile([1, RC], F32, tag="ps")
        nc.tensor.matmul(out=psc[:], lhsT=b1tb[:], rhs=w2t[:],
                         start=True, stop=True)
        crow1 = cpool.tile([1, RC], BF16)
        nc.scalar.copy(out=crow1[:], in_=psc[:])
        onesb = cpool.tile([1, 128], BF16)
        nc.vector.memset(onesb[:], 1.0)
        pscb = pp.tile([128, RC], F32, tag="ps")
        nc.tensor.matmul(out=pscb[:], lhsT=onesb[:], rhs=crow1[:],
                         start=True, stop=True)
        crow128 = cpool.tile([128, RC], F32)
        nc.scalar.copy(out=crow128[:], in_=pscb[:])

        # ---------- dense layer 1: y1[(m*R+r) row] = dinv[m]*(x[m] @ W1_r)
        uT = bigp.tile([128, KC, cfg.NLOC], BF16)
        nc.sync.dma_start(out=uT[:],
                          in_=xT.rearrange("(c p) n -> p c n", p=128))
        y1s = bigp.tile([128, MC, R, H], BF16)
        for mc in range(MC):
            for r in range(R):
                ps = pp.tile([128, H], F32, tag="ps")
                for kc in range(KC):
                    nc.tensor.matmul(
                        out=ps[:],
                        lhsT=uT[:, kc, mc * 128:(mc + 1) * 128],
                        rhs=w1t[:, r * KC + kc, :],
                        start=(kc == 0), stop=(kc == KC - 1))
                nc.scalar.mul(out=y1s[:, mc, r, :], in_=ps[:],
                              mul=dinv[:, mc:mc + 1])
        y1d = dram.tile([MC * 128 * R, H], BF16)
        nc.sync.dma_start(
            out=y1d.rearrange("(m p r) h -> p m r h", p=128, r=R),
            in_=y1s[:])

        LIMIT = int(os.environ.get("KLIMIT", "6"))
        if LIMIT < 2:
            return

        def agg_pass(table_ap, width, part_dram, GT, evac):
            """Gather + one-hot matmul segment sum; staged group writes."""
            partv = part_dram.rearrange("(t p) w -> p t w", p=128)
            c = 0
            s3 = None
            g = None
            stage = None
            for t in range(NT):
                if t % GT == 0:
                    stage = stpool.tile([128, GT, width], BF16,
                                        tag=f"stg{GT}_{width}")
                ps = pp.tile([128, width], F32, tag="ps")
                for j in range(chunks_t[t]):
                    if c % B == 0:
                        b = c // B
                        g = gpool.tile([128, B, 128], BF16, tag="g")
                        nc.gpsimd.dma_gather(
                            out_ap=g[:], in_ap=table_ap,
                            idxs_ap=idxt[:, b * B8:(b + 1) * B8],
                            num_idxs=B * 128, num_idxs_reg=B * 128,
                            elem_size=128)
                    if c % J == 0:
                        s3 = spool.tile([128, J, 128], BF16, tag="s3")
                        nj = min(J, CHpad - c)
                        nc.vector.tensor_tensor(
                            out=s3[:, :nj, :],
                            in0=dloct[:, c:c + nj, :].to_broadcast(
                                [128, nj, 128]),
                            in1=iotab[:].to_broadcast([128, nj, 128]),
                            op=ALU.is_equal)
                    nc.tensor.matmul(
                        out=ps[:], lhsT=s3[:, c % J, :],
                        rhs=g[:, c % B, :width],
                        start=(j == 0), stop=(j == chunks_t[t] - 1))
                    c += 1
                evac(stage[:, t % GT, :], ps)
                if t % GT == GT - 1:
                    t0 = t - GT + 1
                    nc.sync.dma_start(out=partv[:, t0:t0 + GT, :],
                                      in_=stage[:])

        def evac_dve(dst, ps):
            nc.vector.tensor_copy(out=dst, in_=ps[:])

        def evac_act(dst, ps):
            nc.scalar.copy(out=dst, in_=ps[:])

        # ---------- layer-1 aggregation + reduce-scatter (bf16)
        t1p = dram.tile([cfg.N_PAD, H], BF16)
        t1r = dram.tile([cfg.NLOC, H], BF16)
        agg_pass(y1d[:], H, t1p, cfg.GT1, evac_dve)
        if LIMIT < 3:
            return
        nc.gpsimd.collective_compute(
            "ReduceScatter", ALU.add,
            replica_groups=[list(range(cfg.ncores))],
            ins=[t1p.opt()], outs=[t1r.opt()])
        if LIMIT < 4:
            return

        # ---------- layer-2 dense: y2 rows (m*R+r), cols 0:C used
        t1rs = bigp.tile([128, MC, H], BF16)
        nc.sync.dma_start(out=t1rs[:],
                          in_=t1r.rearrange("(m p) h -> p m h", p=128))
        u2T = bigp.tile([128, cfg.NLOC], BF16)
        for mc in range(MC):
            tt = stpool.tile([128, H], BF16, tag="tt")
            nc.vector.tensor_scalar(out=tt[:], in0=t1rs[:, mc, :],
                                    scalar1=dinv2[:, mc:mc + 1],
                                    scalar2=None, op0=ALU.mult)
            pst = pp.tile([128, 128], BF16, tag="ps")
            nc.tensor.transpose(out=pst[:], in_=tt[:], identity=identb[:])
            nc.vector.tensor_copy(out=u2T[:, mc * 128:(mc + 1) * 128],
                                  in_=pst[:])
        y2s = bigp.tile([128, MC, RC], BF16)
        for mc in range(MC):
            ps2 = pp.tile([128, RC], F32, tag="ps")
            nc.tensor.matmul(out=ps2[:],
                             lhsT=u2T[:, mc * 128:(mc + 1) * 128],
                             rhs=w2t[:], start=True, stop=True)
            bias = stpool.tile([128, RC], F32, tag="bias")
            nc.vector.tensor_scalar(out=bias[:], in0=crow128[:],
                                    scalar1=dinv[:, mc:mc + 1],
                                    scalar2=None, op0=ALU.mult)
            nc.vector.tensor_tensor(out=y2s[:, mc, :], in0=ps2[:],
                                    in1=bias[:], op=ALU.add)
        y2d = dram.tile([MC * 128 * R, 128], BF16)
        y2dv = y2d.rearrange("(m p r) h -> p m r h", p=128, r=R)
        for r in range(R):
            nc.sync.dma_start(out=y2dv[:, :, r, 0:C],
                              in_=y2s[:, :, r * C:(r + 1) * C])
        if LIMIT < 5:
            return

        # ---------- layer-2 aggregation + reduce-scatter
        t2p = dram.tile([cfg.N_PAD, C], BF16)
        t2r = dram.tile([cfg.NLOC, C], BF16)
        agg_pass(y2d[:], C, t2p, cfg.GT2, evac_act)
        nc.gpsimd.collective_compute(
            "ReduceScatter", ALU.add,
            replica_groups=[list(range(cfg.ncores))],
            ins=[t2p.opt()], outs=[t2r.opt()])
        if LIMIT < 6:
            return

        # ---------- final: h2 = dinv*t2 + b2 ; fused log_softmax
        t2s = bigp.tile([128, MC, C], BF16)
        nc.sync.dma_start(out=t2s[:],
                          in_=t2r.rearrange("(m p) c -> p m c", p=128))
        ft = bigp.tile([128, MC, C], F32)
        nc.vector.tensor_tensor(
            out=ft[:], in0=t2s[:],
            in1=dinv[:].unsqueeze(2).to_broadcast([128, MC, C]), op=ALU.mult)
        nc.vector.tensor_tensor(
            out=ft[:], in0=ft[:],
            in1=b2t[:].unsqueeze(1).to_broadcast([128, MC, C]), op=ALU.add)
        negmx = bigp.tile([128, MC], F32)
        nc.vector.tensor_reduce(out=negmx[:], in_=ft[:],
                                axis=mybir.AxisListType.X,
                                op=ALU.max, negate=True)
        nc.vector.tensor_tensor(
            out=ft[:], in0=ft[:],
            in1=negmx[:].unsqueeze(2).to_broadcast([128, MC, C]), op=ALU.add)
        ex = bigp.tile([128, MC, C], F32)
        nc.scalar.activation(out=ex[:], in_=ft[:], func=AF.Exp)
        ssum = bigp.tile([128, MC], F32)
        nc.vector.tensor_reduce(out=ssum[:], in_=ex[:],
                                axis=mybir.AxisListType.X, op=ALU.add)
        lg = bigp.tile([128, MC], F32)
        nc.scalar.activation(out=lg[:], in_=ssum[:], func=AF.Ln)
        nc.vector.tensor_tensor(
            out=ft[:], in0=ft[:],
            in1=lg[:].unsqueeze(2).to_broadcast([128, MC, C]),
            op=ALU.subtract)
        nc.sync.dma_start(out=out.rearrange("(m p) c -> p m c", p=128),
                          in_=ft[:])


# ------------------------------------------------------------------ runtime
_PROGRAM_CACHE = {}


def run(cfg, inputs):
    in_maps, chunks_t, CHpad = preprocess(cfg, **inputs)
    key = (cfg.N, cfg.E, chunks_t, CHpad)
    if key not in _PROGRAM_CACHE:
        _PROGRAM_CACHE[key] = build_program(cfg, chunks_t, CHpad)
    nc = _PROGRAM_CACHE[key]
    res = None
    for attempt in range(3):
        try:
            res = run_bass_kernel_spmd(nc, in_maps,
                                       core_ids=list(range(cfg.ncores)))
            break
        except Exception:
            if attempt == 2:
                raise
    outs = [np.asarray(res.results[k]["out"]) for k in range(cfg.ncores)]
    return np.ascontiguousarray(assemble(cfg, outs).astype(np.float32))


def kernel(x, edge_index, edge_relation, W1, b1, W2, b2):
    return run(CFG, dict(x=x, edge_index=edge_index,
                         edge_relation=edge_relation,
                         W1=W1, b1=b1, W2=W2, b2=b2))
